# revision 20
# baseline (speedup 1.0000x reference)
"""Complex dot-product attention on 8 Trainium2 NeuronCores.

Problem (hardcoded shapes): B=4, Q=4096, K=4096, D=64, V=64, complex inputs
stored as [..., 2] (real/imag interleaved, innermost).

    Sr = (Qr Kr^T + Qi Ki^T)/sqrt(D);  Si = (Qr Ki^T - Qi Kr^T)/sqrt(D)
    norm = |S|;  change = softmax(norm, k) / (norm + eps)
    A = S * change;  Y = A @ V (complex)

Sharding: batch (4) x query-halves (2) -> 8 cores; K/V replicated per batch.

Per-core algorithm (S^T layout, k on partitions):
  - flatten (D,2) -> 128-wide contraction: Sr = q~ . k~, Si = q~rot . k~
    with q~rot = rot(q~) ((r,i) -> (-i, r) pairwise).
  - matmul1: S^T tiles [128k, 512q] = kT.T @ qT (f32r, full PE rate)
  - n2 = Sr^2+Si^2 (custom DVE op), nraw = sqrt(n2) (ACT),
    e = exp(nraw/8) (ACT), rn = 1/nraw (DVE approx), t = e*rn (GPSIMD)
  - A^T = S^T * t (DVE x2) -> matmul2: Y^T [128vc, 512q] += V~[j].T-free @ A^T
    (V stationary natural layout, Vrot for imag part), accumulated over k
  - denominator: ones-matmul accumulates sum_k e -> [1, 512]
  - epilogue: Y^T * (1/sum) broadcast, PE-transpose back to [q, vc], DMA out.
"""

import os
import tempfile

import numpy as np

import act_tables as _act_tables_mod  # noqa: F401  (inlined below for grading)

import concourse.bass as bass
import concourse.tile as tile
from concourse import bacc, mybir
from concourse.bass_utils import run_bass_kernel_spmd
from concourse.masks import make_identity

# ---- custom activation tables (exp->E(y), tanh->H(y), single table set)
_ACT_DIR = os.path.join(tempfile.gettempdir(), "act_custom_kernel")
_ACT_PATH, _ACT_DIGEST = _act_tables_mod.generate(_ACT_DIR)
os.environ["BASS_ACT_ROOT_JSON_PATH"] = _ACT_PATH

F32 = mybir.dt.float32
F32R = mybir.dt.float32r
AF = mybir.ActivationFunctionType

B, Q, KK, D, V = 4, 4096, 4096, 64, 64
FEAT = 2 * D          # 128: flattened (d, comp) contraction width
VC = 2 * V            # 128: flattened (v, comp) output width
N_CORES = 8
QSH = Q * B // N_CORES  # 2048 queries per core
QCHUNK = 512
N_CHUNKS = QSH // QCHUNK          # 4
N_KT = KK // 128                  # 32 k-tiles
ACT_GROUP = 8                     # k-tiles per ACT table phase


# ---------------------------------------------------------------- custom DVE op
_CMAG2 = None


def _get_cmag2():
    """Register (once) a custom DVE op: out = in0^2 + in1^2 in a single pass."""
    global _CMAG2
    if _CMAG2 is not None:
        return _CMAG2
    import concourse.dve_ops as dve_ops
    from concourse.dve_spec import Spec, Src0, Src1, sq, lower
    from concourse.dve_uop import DveOpSpec

    name = "CMAG2_ANT"
    if name in dve_ops._SUB_OPCODE_FOR_NAME:
        _CMAG2 = next(op for op in dve_ops.OPS if op.name == name)
        return _CMAG2
    spec = Spec(
        body=sq(Src0) + sq(Src1),
        reference=lambda in0, in1, s0, s1, imm2: (
            in0.astype(np.float32) ** 2 + in1.astype(np.float32) ** 2
        ),
    )
    row = dve_ops._CUSTOM_DVE_ROW_BASE + len(dve_ops.OPS)
    assert row < 0x20
    dve_ops._SUB_OPCODE_FOR_NAME[name] = row
    shas = {}
    for ver in ("v3", "v4"):
        s = DveOpSpec(name=name, opcode=row, uops=lower(spec, ver=ver), rd1_en=True)
        shas[ver] = s.sha(ver)
    op = dve_ops.DveOp(name, spec, subdim=False, uops_sha=shas)
    dve_ops.OPS.append(op)
    dve_ops.CUSTOM_DVE_SPECS[name] = spec
    _CMAG2 = op
    return op


# ------------------------------------------------------------------ bass kernel
def _rot_pairs(nc, dst, src, scale_even=-1.0):
    """dst[:, 2m] = -src[:, 2m+1]; dst[:, 2m+1] = src[:, 2m] (pairwise i*z).
    On GPSIMD (SBUF-only) to keep ACT free for the hot loop."""
    d3 = dst.rearrange("p (m c) -> p m c", c=2)
    s3 = src.rearrange("p (m c) -> p m c", c=2)
    nc.gpsimd.tensor_scalar_mul(d3[:, :, 0:1], s3[:, :, 1:2], scale_even)
    nc.gpsimd.tensor_copy(d3[:, :, 1:2], s3[:, :, 0:1])


def build_nc():
    cmag2 = _get_cmag2()
    nc = bacc.Bacc("TRN2", target_bir_lowering=False, debug=False)
    # digest in the input name busts the neuron compile cache when the
    # activation-table binaries (not part of the BIR) change
    q_d = nc.dram_tensor(f"q_{_ACT_DIGEST}", [QSH, FEAT], F32, kind="ExternalInput")
    k_d = nc.dram_tensor("k", [KK, FEAT], F32, kind="ExternalInput")
    v_d = nc.dram_tensor("v", [KK, VC], F32, kind="ExternalInput")
    y_d = nc.dram_tensor("y", [QSH, VC], F32, kind="ExternalOutput")
    q_ap, k_ap, v_ap, y_ap = q_d.ap(), k_d.ap(), v_d.ap(), y_d.ap()

    with tile.TileContext(nc) as tc:
        with (
            tc.tile_pool(name="const", bufs=1) as constp,
            tc.tile_pool(name="kv", bufs=1) as kvp,
            tc.tile_pool(name="qp", bufs=2) as qp,
            tc.tile_pool(name="ld", bufs=4) as ldp,
            tc.tile_pool(name="st", bufs=5) as stp,
            tc.tile_pool(name="ep", bufs=2) as epp,
            tc.tile_pool(name="ps_s", bufs=5, space="PSUM") as ps_s,
            tc.tile_pool(name="ps_y", bufs=2, space="PSUM") as ps_y,
            tc.tile_pool(name="ps_sum", bufs=1, space="PSUM") as ps_sum,
        ):
            # ---- constants
            ident = constp.tile([128, 128], F32)
            make_identity(nc, ident[:])
            ones_f = constp.tile([128, 1], F32)
            nc.vector.memset(ones_f[:], 1.0)
            ones_col = constp.tile([128, 1], F32R)
            nc.vector.tensor_copy(ones_col[:], ones_f[:])
            onesr_f = constp.tile([1, 128], F32)
            nc.vector.memset(onesr_f[:], 1.0)
            ones_row = constp.tile([1, 128], F32R)
            nc.vector.tensor_copy(ones_row[:], onesr_f[:])

            # ---- per-batch K/V prep (SBUF resident); single big DMAs
            kT_all = kvp.tile([128, KK], F32R)      # feature-major K
            v_all = kvp.tile([128, N_KT * VC], F32R)   # natural V, f32r
            vrot_all = kvp.tile([128, N_KT * VC], F32R)
            k_nat = kvp.tile([128, N_KT, FEAT], F32)   # [p, j, f] natural tiles
            v_nat = kvp.tile([128, N_KT, VC], F32)
            nc.sync.dma_start(k_nat[:], k_ap.rearrange("(j p) f -> p j f", p=128))
            nc.sync.dma_start(v_nat[:], v_ap.rearrange("(j p) f -> p j f", p=128))
            v_nat_flat = v_nat[:].rearrange("p a b -> p (a b)")
            nc.scalar.copy(v_all[:], v_nat_flat)
            _rot_pairs(nc, vrot_all[:], v_nat_flat)
            for j in range(N_KT):
                ktp = ps_s.tile([128, 128], F32, tag="s")
                nc.tensor.transpose(ktp[:], k_nat[:, j, :], ident[:])
                nc.scalar.copy(kT_all[:, j * 128:(j + 1) * 128], ktp[:])

            # ---- per q-chunk stream
            for c in range(N_CHUNKS):
                q0 = c * QCHUNK
                qT = qp.tile([128, QCHUNK], F32R, tag="qT")
                qrotT = qp.tile([128, QCHUNK], F32R, tag="qrotT")
                q_nat = ldp.tile([128, QCHUNK // 128, FEAT], F32, tag="qn")
                nc.sync.dma_start(
                    q_nat[:],
                    q_ap[q0:q0 + QCHUNK, :].rearrange("(t p) f -> p t f", p=128))
                qrotn = ldp.tile([128, QCHUNK // 128, FEAT], F32, tag="qrotn")
                _rot_pairs(nc, qrotn[:].rearrange("p a b -> p (a b)"),
                           q_nat[:].rearrange("p a b -> p (a b)"))
                for t in range(QCHUNK // 128):
                    qtp = ps_s.tile([128, 128], F32, tag="s")
                    nc.tensor.transpose(qtp[:], q_nat[:, t, :], ident[:])
                    nc.scalar.copy(qT[:, t * 128:(t + 1) * 128], qtp[:])
                    qtp2 = ps_s.tile([128, 128], F32, tag="s")
                    nc.tensor.transpose(qtp2[:], qrotn[:, t, :], ident[:])
                    nc.scalar.copy(qrotT[:, t * 128:(t + 1) * 128], qtp2[:])

                yt_ps = ps_y.tile([128, QCHUNK], F32)
                sum_ps = ps_sum.tile([1, QCHUNK], F32)

                # k-tile stream; AF.Tanh/AF.Exp evaluate the custom H/E splines
                for j in range(N_KT):
                    sr = ps_s.tile([128, QCHUNK], F32, tag="s")
                    si = ps_s.tile([128, QCHUNK], F32, tag="s")
                    kT_j = kT_all[:, j * 128:(j + 1) * 128]
                    nc.tensor.matmul(sr[:], kT_j, qT[:], start=True, stop=True)
                    nc.tensor.matmul(si[:], kT_j, qrotT[:], start=True, stop=True)
                    si_sb = stp.tile([128, QCHUNK], F32, tag="si_sb")
                    if j % 2 == 0:
                        nc.scalar.copy(si_sb[:], si[:])
                    else:
                        nc.vector.tensor_copy(si_sb[:], si[:])
                    n2 = stp.tile([128, QCHUNK], F32, tag="n2")
                    nc.vector._custom_dve(cmag2, out=n2[:], in0=sr[:], in1=si_sb[:])
                    h = stp.tile([128, QCHUNK], F32, tag="h")
                    nc.scalar.activation(h[:], n2[:], AF.Tanh)   # H(n2) = e/nraw
                    e = stp.tile([128, QCHUNK], F32R, tag="e")
                    nc.scalar.activation(e[:], n2[:], AF.Exp)    # E(n2) = exp(nraw/8)
                    ar = stp.tile([128, QCHUNK], F32R, tag="ar")
                    nc.vector.tensor_mul(ar[:], sr[:], h[:])
                    ai = stp.tile([128, QCHUNK], F32R, tag="ai")
                    nc.gpsimd.tensor_mul(ai[:], si_sb[:], h[:])
                    v_j = v_all[:, j * VC:(j + 1) * VC]
                    vrot_j = vrot_all[:, j * VC:(j + 1) * VC]
                    nc.tensor.matmul(yt_ps[:], v_j, ar[:], start=(j == 0), stop=False)
                    nc.tensor.matmul(yt_ps[:], vrot_j, ai[:], start=False,
                                     stop=(j == N_KT - 1))
                    nc.tensor.matmul(sum_ps[:], ones_col[:], e[:], start=(j == 0),
                                     stop=(j == N_KT - 1))

                # ---- epilogue: normalize by 1/sum, transpose back, store
                rsum = epp.tile([1, QCHUNK], F32, tag="rsum")
                nc.vector.reciprocal_approx_fast(rsum[:], sum_ps[:])
                rsum_r = epp.tile([1, QCHUNK], F32R, tag="rsum_r")
                nc.vector.tensor_copy(rsum_r[:], rsum[:])
                rsrep = ps_s.tile([128, QCHUNK], F32, tag="s")
                nc.tensor.matmul(rsrep[:], ones_row[:], rsum_r[:], start=True, stop=True)
                rsrep_sb = epp.tile([128, QCHUNK], F32, tag="rsrep_sb")
                nc.scalar.copy(rsrep_sb[:], rsrep[:])
                ytn = epp.tile([128, QCHUNK], F32, tag="ytn")
                nc.vector.tensor_mul(ytn[:], yt_ps[:], rsrep_sb[:])
                yo = epp.tile([128, QCHUNK // 128, VC], F32, tag="yo")
                for t in range(QCHUNK // 128):
                    tr = ps_s.tile([128, 128], F32, tag="s")
                    nc.tensor.transpose(tr[:], ytn[:, t * 128:(t + 1) * 128], ident[:])
                    nc.scalar.copy(yo[:, t, :], tr[:])
                nc.sync.dma_start(
                    y_ap[q0:q0 + QCHUNK, :].rearrange("(t p) f -> p t f", p=128),
                    yo[:])

    nc.compile()
    return nc


# ------------------------------------------------------------------- execution
_CACHED = None


def _get_runner():
    global _CACHED
    if _CACHED is None:
        _CACHED = build_nc()
    return _CACHED


def _shard_inputs(queries, keys, values):
    in_maps = []
    for c in range(N_CORES):
        b, h = c // 2, c % 2
        in_maps.append({
            f"q_{_ACT_DIGEST}": np.ascontiguousarray(
                queries[b, h * QSH:(h + 1) * QSH].reshape(QSH, FEAT)),
            "k": np.ascontiguousarray(keys[b].reshape(KK, FEAT)),
            "v": np.ascontiguousarray(values[b].reshape(KK, VC)),
        })
    return in_maps


def kernel(queries, keys, values):
    queries = np.asarray(queries, dtype=np.float32)
    keys = np.asarray(keys, dtype=np.float32)
    values = np.asarray(values, dtype=np.float32)
    nc = _get_runner()
    in_maps = _shard_inputs(queries, keys, values)
    res = run_bass_kernel_spmd(nc, in_maps, core_ids=list(range(N_CORES)))
    out = np.empty((B, Q, V, 2), dtype=np.float32)
    for c in range(N_CORES):
        b, h = c // 2, c % 2
        out[b, h * QSH:(h + 1) * QSH] = res.results[c]["y"].reshape(QSH, V, 2)
    return out
